# revision 65
# baseline (speedup 1.0000x reference)
"""AttnBlock (GroupNorm -> QKV 1x1 -> full NxN attention -> out-proj + residual)
on 8 Trainium2 NeuronCores, data-parallel over batch (1 batch element/core).

Shapes (hardcoded): x [8, 256, 64, 64] f32, c=256, n=h*w=4096.

Per-core scheme (all on-chip, no transposes):
  - x, q, k live in [c, n] layout: SBUF [128 part, 2 c-chunks, 4096].
    Projections read an f32r rounding copy of x (the BIR verifier requires
    f32r matmul inputs from a rounding producer); GroupNorm's affine is
    folded into the weights/biases on-chip: A,B derived from bn_stats,
    folded into w and the eviction biases.
  - scores computed transposed: sT[j, i] = sum_c k[c,j] q[c,i] (lhsT=k,
    rhs=q), softmax without max-subtraction (scores ~N(0,1) after the 1/16
    scale folded into wq), exp on ScalarE PSUM->SBUF (2 j-tiles per ACT op).
  - v is produced pre-transposed with the output projection folded in:
    v'T[j, co] = sum_ci hn[ci, j] * (wo@wv).T[ci, co]  (lhsT=x, rhs=wovT),
    two j-tiles batched per PSUM bank so evictions are [128,512].
  - q is projected per i-block inside the attention loop, software-pipelined
    one block ahead so its eviction never stalls the score matmuls.
  - out[co, i] = sum_j v'T[j, co] * exp(sT)[j, i] accumulates in PSUM over j;
    softmax denominators via partial sums of exp tiles (5 groups on GpSimd,
    10 on VectorE with the GpSimd partial folded in mid-block, the last
    group fed directly), then ones[128,128] f32r matmuls that reduce over
    partitions and broadcast to 128 partitions; division by the denominator
    + residual add follow on VectorE.
  - the next block's first two score groups are pre-emitted ahead of the
    denominator matmuls so exp latency never stalls the PE at block
    boundaries.
  - biases: bq, bk applied at PSUM eviction (per-partition); bv/bo folded
    host-side into bo' = bo + wo@bv (softmax rows sum to 1), added into the
    x residual per i-block on GpSimd once q has consumed that slice.
  - PE warm-up: dummy matmuls spread across the x-DMA window (plus a batch
    riding the w_l DMA) keep the HAM clock gate at 8/8 so the projection
    matmuls run at 2.4 GHz from the start.
"""

import sys

if "/opt/trn_rl_repo" not in sys.path:
    sys.path.insert(0, "/opt/trn_rl_repo")

import numpy as np

P = 128
C = 256
CC = C // P          # 2 channel chunks
H = W = 64
N = H * W            # 4096
NJT = N // P         # 32 j-tiles of 128
IB = 512             # i-block (psum bank width)
NIB = N // IB        # 8 i-blocks
JG = 2               # j-tiles per exp group
NGRP = NJT // JG     # exp groups per i-block
G = 8                # groups
EPS = 1e-6

# x DMA chunks: uneven so the final bn_stats (on the GN critical path) is
# short
XCHUNKS = [512] * 7 + [448, 64]
XCH = len(XCHUNKS)
XOFF = [sum(XCHUNKS[:i]) for i in range(XCH + 1)]

_CACHE = {}


def _build():
    import concourse.tile as tile
    from concourse import bacc, mybir
    from concourse.bass_interp import get_hw_module

    f32 = mybir.dt.float32
    f32r = mybir.dt.float32r
    AF = mybir.ActivationFunctionType
    OP = mybir.AluOpType

    nc = bacc.Bacc("TRN2", target_bir_lowering=False, debug=False,
                   enable_asserts=False, num_devices=1)

    x_d = nc.dram_tensor("x", (C, N), f32, kind="ExternalInput").ap()
    ws_d = nc.dram_tensor("wstack", (3, C, C), f32, kind="ExternalInput").ap()
    bs_d = nc.dram_tensor("bstack", (5, C), f32, kind="ExternalInput").ap()
    g_d = nc.dram_tensor("Gm", (CC, P, G), f32, kind="ExternalInput").ap()
    gt_d = nc.dram_tensor("GmT", (CC, G, P), f32, kind="ExternalInput").ap()
    out_d = nc.dram_tensor("out", (C, N), f32, kind="ExternalOutput").ap()

    x_r = x_d.rearrange("(cc p) n -> p cc n", p=P)
    out_r = out_d.rearrange("(cc p) n -> p cc n", p=P)

    # index of the one ACT table set covering every func we use
    # (ln, exp, square, copy, identity)
    from concourse.hw_specs import get_activation_tables
    act_sets = list(get_activation_tables(nc.m.arch))
    LNEXP_SET = act_sets.index("natural_log_exp_and_others")

    with tile.TileContext(nc) as tc:
        with (
            tc.tile_pool(name="const", bufs=1) as const,
            tc.tile_pool(name="data", bufs=1) as data,
            tc.tile_pool(name="ps", bufs=1, space="PSUM") as ps,
        ):
            # preload it once at t=0 so no mid-kernel table switches occur
            _ld = mybir.InstLoadActFuncSet(
                name=nc.get_next_instruction_name(), ins=[], outs=[],
                act_func_set_id=LNEXP_SET)
            nc.scalar.add_instruction(_ld)

            # ---- load x in chunks first (DMA-critical), GN stats overlap ----
            xt = data.tile([P, CC, N], f32, tag="x")
            k_t = data.tile([P, CC, N], f32r, tag="k")
            vp_t = data.tile([P, NJT, C], f32r, tag="vp")
            # f32r copy of x for the projection matmuls (the BIR verifier
            # requires f32r matmul inputs to come from a rounding producer);
            # lives in the whole-kernel pool because q projects per i-block
            # inside the attention loop
            xr_t = data.tile([P, CC, N], f32r, tag="xr")

            def xr(ci, sl):
                return xr_t[:, ci, sl]

            with tc.tile_pool(name="hnp", bufs=1) as hnp:
                ones_l = hnp.tile([P, P], f32, tag="onesl")
                nc.gpsimd.memset(ones_l[:], 1.0)
                ones_t = const.tile([P, P], f32r, tag="ones")
                nc.gpsimd.tensor_copy(ones_t[:], ones_l[:])
                eps_t = const.tile([G, 1], f32, tag="eps")
                nc.gpsimd.memset(eps_t[:], EPS)

                # per-chunk bn_stats records; dummy matmuls ride the later
                # chunks to keep the PE HAM clock gate warm through the DMA
                st6 = hnp.tile([P, CC, XCH, 6], f32, tag="st6")
                warm = ps.tile([P, IB], f32, tag="proj", bufs=2, name="warm")
                for xc in range(XCH):
                    nsl = slice(XOFF[xc], XOFF[xc + 1])
                    nc.sync.dma_start(xt[:, :, nsl], x_r[:, :, nsl])
                    for cc in range(CC):
                        nc.vector.bn_stats(st6[:, cc, xc], xt[:, cc, nsl])
                        if (xc + cc) % 2 == 0:
                            nc.scalar.activation(xr_t[:, cc, nsl],
                                                 xt[:, cc, nsl], AF.Copy)
                        else:
                            nc.vector.tensor_copy(xr_t[:, cc, nsl],
                                                  xt[:, cc, nsl])
                    if xc >= 3 and XCHUNKS[xc] >= C:
                        w_n = min(IB, XCHUNKS[xc])
                        wsl = slice(XOFF[xc], XOFF[xc] + w_n)
                        for _ in range(3):
                            nc.tensor.matmul(warm[:, :w_n], ones_t[:],
                                             xr(0, wsl), start=True, stop=True)

                # ---- constants (after x on the DMA queue; small ones first
                # -- g/gt/b gate the GN chain, w_l only the later fold) ----
                b_t = const.tile([P, 5, CC], f32, tag="bt")
                nc.sync.dma_start(b_t[:], bs_d.rearrange("v (cc p) -> p v cc", p=P))
                g_t = const.tile([P, CC, G], f32, tag="G")
                nc.sync.dma_start(g_t[:], g_d.rearrange("cc p g -> p cc g"))
                gt_t = const.tile([G, CC, P], f32, tag="GT")
                nc.sync.dma_start(gt_t[:], gt_d.rearrange("cc g p -> g cc p"))
                w_l = hnp.tile([P, 3, CC, C], f32, tag="wl")
                nc.sync.dma_start(w_l[:], ws_d.rearrange("w (cc p) o -> p w cc o", p=P))

                w_r = const.tile([P, 3, CC, C], f32r, tag="wr")
                wq_t, wk_t, wov_t = w_r[:, 0], w_r[:, 1], w_r[:, 2]
                bq_t, bk_t, bo_t, gns_t, gnb_t = (b_t[:, v] for v in range(5))

                # bridge the PE warm-up across the stats->fold serial chain:
                # these ride the w_l DMA (~16.6us) so the HAM window never
                # sees >3.4us of PE idle before the k projection starts
                wlr = hnp.tile([P, CC, C], f32r, tag="wlr")
                nc.gpsimd.tensor_copy(wlr[:], w_l[:, 0])
                for _ in range(3):
                    nc.tensor.matmul(warm[:], ones_t[:], wlr[:],
                                     start=True, stop=True)

                # per-channel (mean, E[x^2]) from aggregated bn records;
                # cc0 on VectorE, cc1 on GpSimd to shorten the serial chain
                mv = const.tile([P, CC, 2], f32, tag="mv")
                stc = const.tile([P, CC, 2], f32, tag="stc")
                for cc in range(CC):
                    nc.vector.bn_aggr(mv[:, cc], st6[:, cc])
                    eng = nc.vector if cc == 0 else nc.gpsimd
                    eng.tensor_copy(stc[:, cc, 0:1], mv[:, cc, 0:1])
                    eng.tensor_mul(stc[:, cc, 1:2], mv[:, cc, 0:1],
                                   mv[:, cc, 0:1])
                    eng.tensor_add(stc[:, cc, 1:2], stc[:, cc, 1:2],
                                   mv[:, cc, 1:2])

                # group-reduce per-channel (mean, E[x^2]) straight in PSUM
                # (score-tag slot: keeps both proj slots free for chps)
                gps = ps.tile([G, 2], f32, tag="score", bufs=2, name="gps")
                for cc in range(CC):
                    nc.tensor.matmul(gps[:], g_t[:, cc], stc[:, cc],
                                     start=(cc == 0), stop=(cc == CC - 1))

                # grp cols: 0=mean 1=rstd 2=ex2 3=mean^2 4=var 5=sqrt(var+eps)
                CPG = C // G
                grp = const.tile([G, 6], f32, tag="grp")
                nc.vector.tensor_scalar_mul(grp[:, 0:1], gps[:, 0:1], 1.0 / CPG)
                nc.vector.tensor_scalar_mul(grp[:, 2:3], gps[:, 1:2], 1.0 / CPG)
                nc.vector.tensor_mul(grp[:, 3:4], grp[:, 0:1], grp[:, 0:1])
                nc.vector.tensor_sub(grp[:, 4:5], grp[:, 2:3], grp[:, 3:4])
                # rstd = exp(-0.5*ln(var+eps)): keeps every ACT func in the
                # natural_log_exp set -> a single table load for the kernel
                nc.scalar.activation(grp[:, 5:6], grp[:, 4:5], AF.Ln, bias=eps_t[:])
                nc.scalar.activation(grp[:, 1:2], grp[:, 5:6], AF.Exp, scale=-0.5)

                # broadcast group (mean, rstd) to channels; A/B affine coeffs
                ab = const.tile([P, CC, 2], f32, tag="ab")  # 0=A 1=B
                for cc in range(CC):
                    chps = ps.tile([P, 2], f32, tag="proj", bufs=2, name="chps")
                    nc.tensor.matmul(chps[:], gt_t[:, cc], grp[:, 0:2],
                                     start=True, stop=True)
                    # A = rstd * gn_scale
                    nc.vector.tensor_mul(ab[:, cc, 0:1], chps[:, 1:2],
                                         gns_t[:, cc:cc + 1])
                    # B = gn_bias - mean * A
                    nc.vector.tensor_mul(ab[:, cc, 1:2], chps[:, 0:1], ab[:, cc, 0:1])
                    nc.vector.tensor_sub(ab[:, cc, 1:2], gnb_t[:, cc:cc + 1],
                                         ab[:, cc, 1:2])

                # fold GN affine into the weights: w'[ci,:] = w[ci,:]*A[ci]
                # (k on VectorE -- it gates the first projection; wov/q on
                # GpSimd so VectorE is free for the first k evictions)
                for w in (1, 2, 0):
                    for cc in range(CC):
                        eng = nc.vector if w == 1 else nc.gpsimd
                        eng.tensor_scalar_mul(w_r[:, w, cc],
                                              w_l[:, w, cc],
                                              ab[:, cc, 0:1])

                # fold the w^T @ B terms into the eviction biases (fp32,
                # N=1 matmuls); for wov the term rides the residual bias
                # because softmax rows sum to 1
                bq3 = const.tile([P, CC], f32, tag="bq3")
                bk3 = const.tile([P, CC], f32, tag="bk3")
                bo3 = const.tile([P, CC], f32, tag="bo3")
                for w, (b_in, b_out) in ((1, (bk_t, bk3)), (0, (bq_t, bq3)),
                                         (2, (bo_t, bo3))):
                    for oc in range(CC):
                        bp = ps.tile([P, 1], f32, tag="proj", bufs=2, name="bp")
                        for ci in range(CC):
                            nc.tensor.matmul(bp[:],
                                             w_l[:, w, ci, oc * P:(oc + 1) * P],
                                             ab[:, ci, 1:2],
                                             start=(ci == 0), stop=(ci == CC - 1))
                        nc.vector.tensor_add(b_out[:, oc:oc + 1], bp[:],
                                             b_in[:, oc:oc + 1])

                def emit_q(ib):
                    isl = slice(ib * IB, (ib + 1) * IB)
                    qt = data.tile([P, CC, IB], f32r, tag="qt", bufs=2,
                                   name=f"qt{ib}")
                    for oc in range(CC):
                        pq = ps.tile([P, IB], f32, tag="proj", bufs=2, name="pq")
                        for ci in range(CC):
                            nc.tensor.matmul(
                                pq[:],
                                wq_t[:, ci, oc * P:(oc + 1) * P],
                                xr(ci, isl),
                                start=(ci == 0), stop=(ci == CC - 1))
                        if oc == 0:
                            nc.vector.tensor_scalar_add(qt[:, oc], pq[:],
                                                        bq3[:, oc:oc + 1])
                        else:
                            nc.scalar.activation(qt[:, oc], pq[:], AF.Identity,
                                                 bias=bq3[:, oc:oc + 1])
                    return qt

                # ---- k projection (evictions alternate ScalarE/VectorE) ----
                for oc in range(CC):
                    for ib in range(NIB):
                        isl = slice(ib * IB, (ib + 1) * IB)
                        it = oc * NIB + ib
                        # rotate across both PSUM tags for a 4-slot pipeline
                        if it % 2 == 0:
                            pp = ps.tile([P, IB], f32, tag="proj", bufs=2,
                                         name="pp")
                        else:
                            pp = ps.tile([P, JG, IB], f32, tag="score", bufs=2,
                                         name="pps")[:, 0]
                        for ci in range(CC):
                            nc.tensor.matmul(
                                pp,
                                wk_t[:, ci, oc * P:(oc + 1) * P],
                                xr(ci, isl),
                                start=(ci == 0), stop=(ci == CC - 1))
                        if it % 2 == 0:
                            nc.scalar.activation(k_t[:, oc, isl], pp,
                                                 AF.Identity,
                                                 bias=bk3[:, oc:oc + 1])
                        else:
                            nc.vector.tensor_scalar_add(k_t[:, oc, isl], pp,
                                                        bk3[:, oc:oc + 1])

                # ---- v'T[j, co] = sum_ci hn[ci, j] wovT[ci, co] (no bias),
                # two j-tiles per PSUM bank, evictions alternate engines;
                # q for the first i-block projects mid-phase so its eviction
                # clears the queues before the first score matmuls ----
                qts = {}
                for jp in range(NJT // 2):
                    if jp == NJT // 2 - 4:
                        qts[0] = emit_q(0)
                    if jp % 2 == 0:
                        pv = ps.tile([P, 2, C], f32, tag="proj", bufs=2,
                                     name="pv")[:]
                    else:
                        pv = ps.tile([P, JG, IB], f32, tag="score", bufs=2,
                                     name="pvs")[:, :, 0:C]
                    for t in range(2):
                        jt = 2 * jp + t
                        for ci in range(CC):
                            nc.tensor.matmul(
                                pv[:, t],
                                xr(ci, slice(jt * P, (jt + 1) * P)),
                                wov_t[:, ci, :],
                                start=(ci == 0), stop=(ci == CC - 1))
                    if jp % 2 == 0:
                        nc.vector.tensor_copy(vp_t[:, 2 * jp:2 * jp + 2], pv)
                    else:
                        nc.scalar.activation(vp_t[:, 2 * jp:2 * jp + 2], pv,
                                             AF.Copy)

            with tc.tile_pool(name="work", bufs=1) as work:
                def emit_scores(qt, g):
                    ssg = ps.tile([P, JG, IB], f32, tag="score", bufs=2,
                                  name="ssg")
                    for t in range(JG):
                        jt = g * JG + t
                        for ci in range(CC):
                            nc.tensor.matmul(
                                ssg[:, t],
                                k_t[:, ci, jt * P:(jt + 1) * P],
                                qt[:, ci, :],
                                start=(ci == 0), stop=(ci == CC - 1))
                    return ssg

                pre = None
                for ib in range(NIB):
                    isl = slice(ib * IB, (ib + 1) * IB)
                    qt = qts.pop(ib)
                    # residual base: xt += bo3 for this i-block (safe: q for
                    # this block was projected in the previous iteration)
                    for co in range(CC):
                        nc.gpsimd.tensor_scalar_add(xt[:, co, isl],
                                                    xt[:, co, isl],
                                                    bo3[:, co:co + 1])
                    ob = []
                    for co in range(CC):
                        obt = ps.tile([P, IB], f32, tag="ob", bufs=2,
                                      name=f"ob_{ib}_{co}")
                        ob.append(obt)
                    esa = work.tile([P, JG, IB], f32r, tag="esum", bufs=3,
                                    name="esa")
                    esb = work.tile([P, JG, IB], f32r, tag="esum", bufs=3,
                                    name="esb")

                    # two score groups run ahead so the PE stream never has
                    # an ob matmul queued head-of-line behind an unfinished
                    # exp; for ib>0 they were pre-emitted before the previous
                    # block's denominator matmuls
                    if pre is None:
                        ssgs = {0: emit_scores(qt, 0), 1: emit_scores(qt, 1)}
                    else:
                        ssgs = dict(enumerate(pre))
                    for g in range(NGRP):
                        ssg = ssgs.pop(g)
                        et = work.tile([P, JG, IB], f32r, tag="exp", bufs=4,
                                       name="et")
                        if isinstance(ssg, list):
                            for t in range(JG):
                                nc.scalar.activation(et[:, t], ssg[t][:],
                                                     AF.Exp)
                        else:
                            nc.scalar.activation(et[:], ssg[:], AF.Exp)
                        if g == 0 and ib + 1 < NIB:
                            # project q for the next i-block now; its eviction
                            # completes long before that block's scores start
                            qts[ib + 1] = emit_q(ib + 1)
                        if g + 2 < NGRP and g + 2 not in ssgs:
                            ssgs[g + 2] = emit_scores(qt, g + 2)
                        if g == NGRP - 1:
                            # fold the esa column-halves on VectorE (off the
                            # tail: its input chain ended at g-1) so the
                            # partition-reduce needs one matmul, not two
                            esaT = work.tile([P, IB], f32r, tag="esat",
                                             bufs=2, name="esaT")
                            nc.vector.tensor_add(esaT[:],
                                                 esa[:, 0].bitcast(f32),
                                                 esa[:, 1].bitcast(f32))
                            # denominators FIRST: the ones-matmuls and the ob
                            # group below both wait on this exp, so ordering
                            # the denominator ahead lets the reciprocal run
                            # during the final ob matmuls -- off the tail
                            smt = ps.tile([P, IB], f32, tag="proj", bufs=2,
                                          name="smt")
                            nc.tensor.matmul(smt[:], ones_t[:], esaT[:],
                                             start=True, stop=False)
                            nc.tensor.matmul(smt[:], ones_t[:], et[:, 0],
                                             start=False, stop=False)
                            nc.tensor.matmul(smt[:], ones_t[:], et[:, 1],
                                             start=False, stop=True)
                            rec = work.tile([P, IB], f32, tag="rec", bufs=2,
                                            name="rec")
                            nc.vector.reciprocal(rec[:], smt[:])
                        for t in range(JG):
                            jt = g * JG + t
                            for co in range(CC):
                                nc.tensor.matmul(
                                    ob[co][:],
                                    vp_t[:, jt, co * P:(co + 1) * P],
                                    et[:, t],
                                    start=(jt == 0), stop=(jt == NJT - 1))
                        # partial-sum split: GpSimd is ~2x slower per add than
                        # VectorE, so it gets 5 groups and VectorE 10; the
                        # last group feeds the denominator matmuls directly
                        if g == NGRP - 1:
                            pass
                        elif g == 0:
                            nc.gpsimd.tensor_copy(esb[:], et[:].bitcast(f32))
                        elif g < 5:
                            nc.gpsimd.tensor_add(esb[:], esb[:].bitcast(f32),
                                                 et[:].bitcast(f32))
                        elif g == 5:
                            nc.vector.tensor_copy(esa[:], et[:].bitcast(f32))
                        else:
                            nc.vector.tensor_add(esa[:], esa[:].bitcast(f32),
                                                 et[:].bitcast(f32))
                            if g == 10:
                                # fold the (complete) GpSimd partial into the
                                # VectorE chain here, well off the tail path
                                nc.vector.tensor_add(esa[:],
                                                     esa[:].bitcast(f32),
                                                     esb[:].bitcast(f32))

                    # pre-emit the next block's first two score groups:
                    # exp(0') latency then hides behind this block's tail
                    # instead of stalling the PE.  The final block gets a
                    # third group (split over the two proj-tag banks) because
                    # it has no next-q matmuls to pad the runway with.
                    if ib + 1 < NIB:
                        nqt = qts[ib + 1]
                        pre = [emit_scores(nqt, 0), emit_scores(nqt, 1)]
                        if ib + 1 == NIB - 1:
                            g2 = []
                            for t in range(JG):
                                jt = 2 * JG + t
                                s1 = ps.tile([P, IB], f32, tag="proj", bufs=2,
                                             name="pre2")
                                for ci in range(CC):
                                    nc.tensor.matmul(
                                        s1[:],
                                        k_t[:, ci, jt * P:(jt + 1) * P],
                                        nqt[:, ci, :],
                                        start=(ci == 0), stop=(ci == CC - 1))
                                g2.append(s1)
                            pre.append(g2)
                    else:
                        pre = None

                    # free the ob psum slots right away (one copy per engine);
                    # the last block divides co0 straight from PSUM instead
                    # (no successor needs the bank), keeping its tail short
                    obs = []
                    for co in range(CC):
                        if ib == NIB - 1 and co == 0:
                            obs.append(None)
                            continue
                        ot = work.tile([P, IB], f32, tag="obs", bufs=4,
                                       name=f"obs_{ib}_{co}")
                        if co == 0:
                            nc.scalar.activation(ot[:], ob[co][:], AF.Copy)
                        elif ib == NIB - 1:
                            nc.scalar.activation(ot[:], ob[co][:], AF.Copy)
                        else:
                            nc.vector.tensor_copy(ot[:], ob[co][:])
                        obs.append(ot)

                    for co in range(CC):
                        on_t = work.tile([P, IB], f32, tag="on", bufs=3, name="on_t")
                        fin = work.tile([P, IB], f32, tag="fin", bufs=3, name="fin")
                        if ib == NIB - 1:
                            # last block: co0 divides from PSUM on VectorE,
                            # co1 from SBUF on GpSimd in parallel; the two
                            # stores issue on different DMA queues
                            if co == 0:
                                nc.vector.tensor_mul(on_t[:], ob[co][:], rec[:])
                            else:
                                nc.gpsimd.tensor_mul(on_t[:], obs[co][:], rec[:])
                            nc.vector.tensor_add(fin[:], on_t[:], xt[:, co, isl])
                            if co == 0:
                                nc.sync.dma_start(out_r[:, co, isl], fin[:])
                            else:
                                nc.scalar.dma_start(out_r[:, co, isl], fin[:])
                        else:
                            nc.vector.tensor_mul(on_t[:], obs[co][:], rec[:])
                            nc.vector.tensor_add(fin[:], on_t[:], xt[:, co, isl])
                            nc.sync.dma_start(out_r[:, co, isl], fin[:])

    nc.compile()
    nc.m = get_hw_module(nc.m)
    return nc


def _get_nc():
    if "nc" not in _CACHE:
        _CACHE["nc"] = _build()
    return _CACHE["nc"]


def _prep_inputs(x, gn_scale, gn_bias, wq, bq, wk, bk, wv, bv, wo, bo):
    f = np.float32
    x = np.asarray(x, f)
    b = x.shape[0]
    scale = 1.0 / np.sqrt(np.float64(C))
    wqT = (np.asarray(wq, np.float64) * scale).T
    bq2 = (np.asarray(bq, np.float64) * scale).astype(f)
    wkT = np.asarray(wk, np.float64).T
    wovT = (np.asarray(wo, np.float64) @ np.asarray(wv, np.float64)).T
    bo2 = (np.asarray(bo, np.float64)
           + np.asarray(wo, np.float64) @ np.asarray(bv, np.float64)).astype(f)
    wstack = np.ascontiguousarray(
        np.stack([wqT, wkT, wovT]).astype(f))
    bstack = np.ascontiguousarray(np.stack(
        [bq2, np.asarray(bk, f), bo2, np.asarray(gn_scale, f),
         np.asarray(gn_bias, f)]))

    gm = np.zeros((CC, P, G), f)
    for cc in range(CC):
        for p in range(P):
            gm[cc, p, (cc * P + p) // (C // G)] = 1.0
    gmT = np.ascontiguousarray(np.transpose(gm, (0, 2, 1)))

    shared = {"wstack": wstack, "bstack": bstack, "Gm": gm, "GmT": gmT}
    in_maps = []
    for i in range(b):
        m = dict(shared)
        m["x"] = np.ascontiguousarray(x[i].reshape(C, N))
        in_maps.append(m)
    return in_maps


def _run(in_maps, trace=False, trace_cores=None):
    from concourse import bass_utils
    nc = _get_nc()
    return bass_utils.run_bass_kernel_spmd(
        nc, in_maps, core_ids=list(range(len(in_maps))),
        trace=trace, trace_cores=trace_cores)


def kernel(x, gn_scale, gn_bias, wq, bq, wk, bk, wv, bv, wo, bo):
    in_maps = _prep_inputs(x, gn_scale, gn_bias, wq, bq, wk, bk, wv, bv, wo, bo)
    res = _run(in_maps)
    b = np.asarray(x).shape[0]
    out = np.stack([res.results[i]["out"].reshape(C, H, W) for i in range(b)])
    return out.astype(np.float32)


# revision 66
# speedup vs baseline: 1.0086x; 1.0086x over previous
"""AttnBlock (GroupNorm -> QKV 1x1 -> full NxN attention -> out-proj + residual)
on 8 Trainium2 NeuronCores, data-parallel over batch (1 batch element/core).

Shapes (hardcoded): x [8, 256, 64, 64] f32, c=256, n=h*w=4096.

Per-core scheme (all on-chip, no transposes):
  - x, q, k live in [c, n] layout: SBUF [128 part, 2 c-chunks, 4096].
    Projections read an f32r rounding copy of x (the BIR verifier requires
    f32r matmul inputs from a rounding producer); GroupNorm's affine is
    folded into the weights/biases on-chip: A,B derived from bn_stats,
    folded into w and the eviction biases.
  - scores computed transposed: sT[j, i] = sum_c k[c,j] q[c,i] (lhsT=k,
    rhs=q), softmax without max-subtraction (scores ~N(0,1) after the 1/16
    scale folded into wq), exp on ScalarE PSUM->SBUF (2 j-tiles per ACT op).
  - v is produced pre-transposed with the output projection folded in:
    v'T[j, co] = sum_ci hn[ci, j] * (wo@wv).T[ci, co]  (lhsT=x, rhs=wovT),
    two j-tiles batched per PSUM bank so evictions are [128,512].
  - q is projected per i-block inside the attention loop, software-pipelined
    one block ahead so its eviction never stalls the score matmuls.
  - out[co, i] = sum_j v'T[j, co] * exp(sT)[j, i] accumulates in PSUM over j;
    softmax denominators via partial sums of exp tiles (5 groups on GpSimd,
    10 on VectorE with the GpSimd partial folded in mid-block, the last
    group fed directly), then ones[128,128] f32r matmuls that reduce over
    partitions and broadcast to 128 partitions; division by the denominator
    + residual add follow on VectorE.
  - the next block's first two score groups are pre-emitted ahead of the
    denominator matmuls so exp latency never stalls the PE at block
    boundaries.
  - biases: bq, bk applied at PSUM eviction (per-partition); bv/bo folded
    host-side into bo' = bo + wo@bv (softmax rows sum to 1), added into the
    x residual per i-block on GpSimd once q has consumed that slice.
  - PE warm-up: dummy matmuls spread across the x-DMA window (plus a batch
    riding the w_l DMA) keep the HAM clock gate at 8/8 so the projection
    matmuls run at 2.4 GHz from the start.
"""

import sys

if "/opt/trn_rl_repo" not in sys.path:
    sys.path.insert(0, "/opt/trn_rl_repo")

import numpy as np

P = 128
C = 256
CC = C // P          # 2 channel chunks
H = W = 64
N = H * W            # 4096
NJT = N // P         # 32 j-tiles of 128
IB = 512             # i-block (psum bank width)
NIB = N // IB        # 8 i-blocks
JG = 2               # j-tiles per exp group
NGRP = NJT // JG     # exp groups per i-block
G = 8                # groups
EPS = 1e-6

# x DMA chunks: uneven so the final bn_stats (on the GN critical path) is
# short
XCHUNKS = [512] * 7 + [448, 64]
XCH = len(XCHUNKS)
XOFF = [sum(XCHUNKS[:i]) for i in range(XCH + 1)]

_CACHE = {}


def _build():
    import concourse.tile as tile
    from concourse import bacc, mybir
    from concourse.bass_interp import get_hw_module

    f32 = mybir.dt.float32
    f32r = mybir.dt.float32r
    AF = mybir.ActivationFunctionType
    OP = mybir.AluOpType

    nc = bacc.Bacc("TRN2", target_bir_lowering=False, debug=False,
                   enable_asserts=False, num_devices=1)

    x_d = nc.dram_tensor("x", (C, N), f32, kind="ExternalInput").ap()
    ws_d = nc.dram_tensor("wstack", (3, C, C), f32, kind="ExternalInput").ap()
    bs_d = nc.dram_tensor("bstack", (5, C), f32, kind="ExternalInput").ap()
    g_d = nc.dram_tensor("Gm", (CC, P, G), f32, kind="ExternalInput").ap()
    gt_d = nc.dram_tensor("GmT", (CC, G, P), f32, kind="ExternalInput").ap()
    out_d = nc.dram_tensor("out", (C, N), f32, kind="ExternalOutput").ap()

    x_r = x_d.rearrange("(cc p) n -> p cc n", p=P)
    out_r = out_d.rearrange("(cc p) n -> p cc n", p=P)

    # index of the one ACT table set covering every func we use
    # (ln, exp, square, copy, identity)
    from concourse.hw_specs import get_activation_tables
    act_sets = list(get_activation_tables(nc.m.arch))
    LNEXP_SET = act_sets.index("natural_log_exp_and_others")

    with tile.TileContext(nc) as tc:
        with (
            tc.tile_pool(name="const", bufs=1) as const,
            tc.tile_pool(name="data", bufs=1) as data,
            tc.tile_pool(name="ps", bufs=1, space="PSUM") as ps,
        ):
            # preload it once at t=0 so no mid-kernel table switches occur
            _ld = mybir.InstLoadActFuncSet(
                name=nc.get_next_instruction_name(), ins=[], outs=[],
                act_func_set_id=LNEXP_SET)
            nc.scalar.add_instruction(_ld)

            # ---- load x in chunks first (DMA-critical), GN stats overlap ----
            xt = data.tile([P, CC, N], f32, tag="x")
            k_t = data.tile([P, CC, N], f32r, tag="k")
            vp_t = data.tile([P, NJT, C], f32r, tag="vp")
            # f32r copy of x for the projection matmuls (the BIR verifier
            # requires f32r matmul inputs to come from a rounding producer);
            # lives in the whole-kernel pool because q projects per i-block
            # inside the attention loop
            xr_t = data.tile([P, CC, N], f32r, tag="xr")

            def xr(ci, sl):
                return xr_t[:, ci, sl]

            with tc.tile_pool(name="hnp", bufs=1) as hnp:
                ones_l = hnp.tile([P, P], f32, tag="onesl")
                nc.gpsimd.memset(ones_l[:], 1.0)
                ones_t = const.tile([P, P], f32r, tag="ones")
                nc.gpsimd.tensor_copy(ones_t[:], ones_l[:])
                eps_t = const.tile([G, 1], f32, tag="eps")
                nc.gpsimd.memset(eps_t[:], EPS)

                # per-chunk bn_stats records; dummy matmuls ride the later
                # chunks to keep the PE HAM clock gate warm through the DMA
                st6 = hnp.tile([P, CC, XCH, 6], f32, tag="st6")
                warm = ps.tile([P, IB], f32, tag="proj", bufs=2, name="warm")
                for xc in range(XCH):
                    nsl = slice(XOFF[xc], XOFF[xc + 1])
                    nc.sync.dma_start(xt[:, :, nsl], x_r[:, :, nsl])
                    for cc in range(CC):
                        nc.vector.bn_stats(st6[:, cc, xc], xt[:, cc, nsl])
                        if (xc + cc) % 2 == 0:
                            nc.scalar.activation(xr_t[:, cc, nsl],
                                                 xt[:, cc, nsl], AF.Copy)
                        else:
                            nc.vector.tensor_copy(xr_t[:, cc, nsl],
                                                  xt[:, cc, nsl])
                    if xc >= 3 and XCHUNKS[xc] >= C:
                        w_n = min(IB, XCHUNKS[xc])
                        wsl = slice(XOFF[xc], XOFF[xc] + w_n)
                        for _ in range(3):
                            nc.tensor.matmul(warm[:, :w_n], ones_t[:],
                                             xr(0, wsl), start=True, stop=True)

                # ---- constants (after x on the DMA queue; small ones first
                # -- g/gt/b gate the GN chain, w_l only the later fold) ----
                b_t = const.tile([P, 5, CC], f32, tag="bt")
                nc.sync.dma_start(b_t[:], bs_d.rearrange("v (cc p) -> p v cc", p=P))
                g_t = const.tile([P, CC, G], f32, tag="G")
                nc.sync.dma_start(g_t[:], g_d.rearrange("cc p g -> p cc g"))
                gt_t = const.tile([G, CC, P], f32, tag="GT")
                nc.sync.dma_start(gt_t[:], gt_d.rearrange("cc g p -> g cc p"))
                w_l = hnp.tile([P, 3, CC, C], f32, tag="wl")
                nc.sync.dma_start(w_l[:], ws_d.rearrange("w (cc p) o -> p w cc o", p=P))

                w_r = const.tile([P, 3, CC, C], f32r, tag="wr")
                wq_t, wk_t, wov_t = w_r[:, 0], w_r[:, 1], w_r[:, 2]
                bq_t, bk_t, bo_t, gns_t, gnb_t = (b_t[:, v] for v in range(5))

                # bridge the PE warm-up across the stats->fold serial chain:
                # these ride the w_l DMA (~16.6us) so the HAM window never
                # sees >3.4us of PE idle before the k projection starts
                wlr = hnp.tile([P, CC, C], f32r, tag="wlr")
                nc.gpsimd.tensor_copy(wlr[:], w_l[:, 0])
                for _ in range(3):
                    nc.tensor.matmul(warm[:], ones_t[:], wlr[:],
                                     start=True, stop=True)

                # per-channel (mean, E[x^2]) from aggregated bn records;
                # cc0 on VectorE, cc1 on GpSimd to shorten the serial chain
                mv = const.tile([P, CC, 2], f32, tag="mv")
                stc = const.tile([P, CC, 2], f32, tag="stc")
                for cc in range(CC):
                    nc.vector.bn_aggr(mv[:, cc], st6[:, cc])
                    eng = nc.vector if cc == 0 else nc.gpsimd
                    eng.tensor_copy(stc[:, cc, 0:1], mv[:, cc, 0:1])
                    eng.tensor_mul(stc[:, cc, 1:2], mv[:, cc, 0:1],
                                   mv[:, cc, 0:1])
                    eng.tensor_add(stc[:, cc, 1:2], stc[:, cc, 1:2],
                                   mv[:, cc, 1:2])

                # group-reduce per-channel (mean, E[x^2]) straight in PSUM
                # (score-tag slot: keeps both proj slots free for chps)
                gps = ps.tile([G, 2], f32, tag="score", bufs=2, name="gps")
                for cc in range(CC):
                    nc.tensor.matmul(gps[:], g_t[:, cc], stc[:, cc],
                                     start=(cc == 0), stop=(cc == CC - 1))

                # grp cols: 0=mean 1=rstd 2=ex2 3=mean^2 4=var 5=sqrt(var+eps)
                CPG = C // G
                grp = const.tile([G, 6], f32, tag="grp")
                nc.vector.tensor_scalar_mul(grp[:, 0:1], gps[:, 0:1], 1.0 / CPG)
                nc.vector.tensor_scalar_mul(grp[:, 2:3], gps[:, 1:2], 1.0 / CPG)
                nc.vector.tensor_mul(grp[:, 3:4], grp[:, 0:1], grp[:, 0:1])
                nc.vector.tensor_sub(grp[:, 4:5], grp[:, 2:3], grp[:, 3:4])
                # rstd = exp(-0.5*ln(var+eps)): keeps every ACT func in the
                # natural_log_exp set -> a single table load for the kernel
                nc.scalar.activation(grp[:, 5:6], grp[:, 4:5], AF.Ln, bias=eps_t[:])
                nc.scalar.activation(grp[:, 1:2], grp[:, 5:6], AF.Exp, scale=-0.5)

                # broadcast group (mean, rstd) to channels; A/B affine coeffs
                ab = const.tile([P, CC, 2], f32, tag="ab")  # 0=A 1=B
                for cc in range(CC):
                    chps = ps.tile([P, 2], f32, tag="proj", bufs=2, name="chps")
                    nc.tensor.matmul(chps[:], gt_t[:, cc], grp[:, 0:2],
                                     start=True, stop=True)
                    # A = rstd * gn_scale
                    nc.vector.tensor_mul(ab[:, cc, 0:1], chps[:, 1:2],
                                         gns_t[:, cc:cc + 1])
                    # B = gn_bias - mean * A
                    nc.vector.tensor_mul(ab[:, cc, 1:2], chps[:, 0:1], ab[:, cc, 0:1])
                    nc.vector.tensor_sub(ab[:, cc, 1:2], gnb_t[:, cc:cc + 1],
                                         ab[:, cc, 1:2])

                # fold GN affine into the weights: w'[ci,:] = w[ci,:]*A[ci]
                # (k on VectorE -- it gates the first projection; wov/q on
                # GpSimd so VectorE is free for the first k evictions)
                for w in (1, 2, 0):
                    for cc in range(CC):
                        eng = nc.vector if w == 1 else nc.gpsimd
                        eng.tensor_scalar_mul(w_r[:, w, cc],
                                              w_l[:, w, cc],
                                              ab[:, cc, 0:1])

                # fold the w^T @ B terms into the eviction biases (fp32,
                # N=1 matmuls); for wov the term rides the residual bias
                # because softmax rows sum to 1
                bq3 = const.tile([P, CC], f32, tag="bq3")
                bk3 = const.tile([P, CC], f32, tag="bk3")
                bo3 = const.tile([P, CC], f32, tag="bo3")
                for w, (b_in, b_out) in ((1, (bk_t, bk3)), (0, (bq_t, bq3)),
                                         (2, (bo_t, bo3))):
                    for oc in range(CC):
                        bp = ps.tile([P, 1], f32, tag="proj", bufs=2, name="bp")
                        for ci in range(CC):
                            nc.tensor.matmul(bp[:],
                                             w_l[:, w, ci, oc * P:(oc + 1) * P],
                                             ab[:, ci, 1:2],
                                             start=(ci == 0), stop=(ci == CC - 1))
                        nc.vector.tensor_add(b_out[:, oc:oc + 1], bp[:],
                                             b_in[:, oc:oc + 1])

                def emit_q(ib):
                    isl = slice(ib * IB, (ib + 1) * IB)
                    qt = data.tile([P, CC, IB], f32r, tag="qt", bufs=2,
                                   name=f"qt{ib}")
                    for oc in range(CC):
                        pq = ps.tile([P, IB], f32, tag="proj", bufs=2, name="pq")
                        for ci in range(CC):
                            nc.tensor.matmul(
                                pq[:],
                                wq_t[:, ci, oc * P:(oc + 1) * P],
                                xr(ci, isl),
                                start=(ci == 0), stop=(ci == CC - 1))
                        if oc == 0:
                            nc.vector.tensor_scalar_add(qt[:, oc], pq[:],
                                                        bq3[:, oc:oc + 1])
                        else:
                            nc.scalar.activation(qt[:, oc], pq[:], AF.Identity,
                                                 bias=bq3[:, oc:oc + 1])
                    return qt

                # ---- k projection (evictions alternate ScalarE/VectorE) ----
                for oc in range(CC):
                    for ib in range(NIB):
                        isl = slice(ib * IB, (ib + 1) * IB)
                        it = oc * NIB + ib
                        # rotate across both PSUM tags for a 4-slot pipeline
                        if it % 2 == 0:
                            pp = ps.tile([P, IB], f32, tag="proj", bufs=2,
                                         name="pp")
                        else:
                            pp = ps.tile([P, JG, IB], f32, tag="score", bufs=2,
                                         name="pps")[:, 0]
                        for ci in range(CC):
                            nc.tensor.matmul(
                                pp,
                                wk_t[:, ci, oc * P:(oc + 1) * P],
                                xr(ci, isl),
                                start=(ci == 0), stop=(ci == CC - 1))
                        if it % 2 == 0:
                            nc.scalar.activation(k_t[:, oc, isl], pp,
                                                 AF.Identity,
                                                 bias=bk3[:, oc:oc + 1])
                        else:
                            nc.vector.tensor_scalar_add(k_t[:, oc, isl], pp,
                                                        bk3[:, oc:oc + 1])

                # ---- v'T[j, co] = sum_ci hn[ci, j] wovT[ci, co] (no bias),
                # two j-tiles per PSUM bank, evictions alternate engines;
                # q for the first i-block projects mid-phase so its eviction
                # clears the queues before the first score matmuls ----
                qts = {}
                for jp in range(NJT // 2):
                    if jp == NJT // 2 - 4:
                        qts[0] = emit_q(0)
                    if jp % 2 == 0:
                        pv = ps.tile([P, 2, C], f32, tag="proj", bufs=2,
                                     name="pv")[:]
                    else:
                        pv = ps.tile([P, JG, IB], f32, tag="score", bufs=2,
                                     name="pvs")[:, :, 0:C]
                    for t in range(2):
                        jt = 2 * jp + t
                        for ci in range(CC):
                            nc.tensor.matmul(
                                pv[:, t],
                                xr(ci, slice(jt * P, (jt + 1) * P)),
                                wov_t[:, ci, :],
                                start=(ci == 0), stop=(ci == CC - 1))
                    if jp % 2 == 0:
                        nc.vector.tensor_copy(vp_t[:, 2 * jp:2 * jp + 2], pv)
                    else:
                        nc.scalar.activation(vp_t[:, 2 * jp:2 * jp + 2], pv,
                                             AF.Copy)

            with tc.tile_pool(name="work", bufs=1) as work:
                def emit_scores(qt, g):
                    ssg = ps.tile([P, JG, IB], f32, tag="score", bufs=2,
                                  name="ssg")
                    for t in range(JG):
                        jt = g * JG + t
                        for ci in range(CC):
                            nc.tensor.matmul(
                                ssg[:, t],
                                k_t[:, ci, jt * P:(jt + 1) * P],
                                qt[:, ci, :],
                                start=(ci == 0), stop=(ci == CC - 1))
                    return ssg

                pre = None
                for ib in range(NIB):
                    isl = slice(ib * IB, (ib + 1) * IB)
                    qt = qts.pop(ib)
                    # residual base: xt += bo3 for this i-block (safe: q for
                    # this block was projected in the previous iteration)
                    for co in range(CC):
                        nc.gpsimd.tensor_scalar_add(xt[:, co, isl],
                                                    xt[:, co, isl],
                                                    bo3[:, co:co + 1])
                    ob = []
                    for co in range(CC):
                        obt = ps.tile([P, IB], f32, tag="ob", bufs=2,
                                      name=f"ob_{ib}_{co}")
                        ob.append(obt)
                    esa = work.tile([P, JG, IB], f32r, tag="esum", bufs=3,
                                    name="esa")
                    esb = work.tile([P, JG, IB], f32r, tag="esum", bufs=3,
                                    name="esb")

                    # two score groups run ahead so the PE stream never has
                    # an ob matmul queued head-of-line behind an unfinished
                    # exp; for ib>0 they were pre-emitted before the previous
                    # block's denominator matmuls
                    if pre is None:
                        ssgs = {0: emit_scores(qt, 0), 1: emit_scores(qt, 1)}
                    else:
                        ssgs = dict(enumerate(pre))
                    for g in range(NGRP):
                        ssg = ssgs.pop(g)
                        et = work.tile([P, JG, IB], f32r, tag="exp", bufs=4,
                                       name="et")
                        if isinstance(ssg, list):
                            for t in range(JG):
                                nc.scalar.activation(et[:, t], ssg[t][:],
                                                     AF.Exp)
                        else:
                            nc.scalar.activation(et[:], ssg[:], AF.Exp)
                        if g == 0 and ib + 1 < NIB:
                            # project q for the next i-block now; its eviction
                            # completes long before that block's scores start
                            qts[ib + 1] = emit_q(ib + 1)
                        if g + 2 < NGRP and g + 2 not in ssgs:
                            ssgs[g + 2] = emit_scores(qt, g + 2)
                        if g == NGRP - 1:
                            # fold the esa column-halves on VectorE (off the
                            # tail: its input chain ended at g-1) so the
                            # partition-reduce needs fewer matmuls
                            esaT = work.tile([P, IB], f32r, tag="esat",
                                             bufs=2, name="esaT")
                            nc.vector.tensor_add(esaT[:],
                                                 esa[:, 0].bitcast(f32),
                                                 esa[:, 1].bitcast(f32))
                            # denominators FIRST: the ones-matmuls and the ob
                            # group below both wait on this exp, so ordering
                            # the denominator ahead lets the reciprocal run
                            # during the final ob matmuls -- off the tail
                            smt = ps.tile([P, IB], f32, tag="proj", bufs=2,
                                          name="smt")
                            if ib < NIB - 1:
                                # mid-kernel blocks have ~28us of tail slack:
                                # fold this exp group's halves on VectorE too
                                # and reduce with a single ones-matmul
                                nc.vector.tensor_add(esaT[:],
                                                     esaT[:].bitcast(f32),
                                                     et[:, 0].bitcast(f32))
                                nc.vector.tensor_add(esaT[:],
                                                     esaT[:].bitcast(f32),
                                                     et[:, 1].bitcast(f32))
                                nc.tensor.matmul(smt[:], ones_t[:], esaT[:],
                                                 start=True, stop=True)
                            else:
                                # final block: exp latency must not extend the
                                # exposed tail -- feed its halves directly
                                nc.tensor.matmul(smt[:], ones_t[:], esaT[:],
                                                 start=True, stop=False)
                                nc.tensor.matmul(smt[:], ones_t[:], et[:, 0],
                                                 start=False, stop=False)
                                nc.tensor.matmul(smt[:], ones_t[:], et[:, 1],
                                                 start=False, stop=True)
                            rec = work.tile([P, IB], f32, tag="rec", bufs=2,
                                            name="rec")
                            nc.vector.reciprocal(rec[:], smt[:])
                        for t in range(JG):
                            jt = g * JG + t
                            for co in range(CC):
                                nc.tensor.matmul(
                                    ob[co][:],
                                    vp_t[:, jt, co * P:(co + 1) * P],
                                    et[:, t],
                                    start=(jt == 0), stop=(jt == NJT - 1))
                        # partial-sum split: GpSimd is ~2x slower per add than
                        # VectorE, so it gets 5 groups and VectorE 10; the
                        # last group feeds the denominator matmuls directly
                        if g == NGRP - 1:
                            pass
                        elif g == 0:
                            nc.gpsimd.tensor_copy(esb[:], et[:].bitcast(f32))
                        elif g < 5:
                            nc.gpsimd.tensor_add(esb[:], esb[:].bitcast(f32),
                                                 et[:].bitcast(f32))
                        elif g == 5:
                            nc.vector.tensor_copy(esa[:], et[:].bitcast(f32))
                        else:
                            nc.vector.tensor_add(esa[:], esa[:].bitcast(f32),
                                                 et[:].bitcast(f32))
                            if g == 10:
                                # fold the (complete) GpSimd partial into the
                                # VectorE chain here, well off the tail path
                                nc.vector.tensor_add(esa[:],
                                                     esa[:].bitcast(f32),
                                                     esb[:].bitcast(f32))

                    # pre-emit the next block's first two score groups:
                    # exp(0') latency then hides behind this block's tail
                    # instead of stalling the PE.  The final block gets a
                    # third group (split over the two proj-tag banks) because
                    # it has no next-q matmuls to pad the runway with.
                    if ib + 1 < NIB:
                        nqt = qts[ib + 1]
                        pre = [emit_scores(nqt, 0), emit_scores(nqt, 1)]
                        if ib + 1 == NIB - 1:
                            g2 = []
                            for t in range(JG):
                                jt = 2 * JG + t
                                s1 = ps.tile([P, IB], f32, tag="proj", bufs=2,
                                             name="pre2")
                                for ci in range(CC):
                                    nc.tensor.matmul(
                                        s1[:],
                                        k_t[:, ci, jt * P:(jt + 1) * P],
                                        nqt[:, ci, :],
                                        start=(ci == 0), stop=(ci == CC - 1))
                                g2.append(s1)
                            pre.append(g2)
                    else:
                        pre = None

                    # free the ob psum slots right away (one copy per engine);
                    # the last block divides co0 straight from PSUM instead
                    # (no successor needs the bank), keeping its tail short
                    obs = []
                    for co in range(CC):
                        if ib == NIB - 1 and co == 0:
                            obs.append(None)
                            continue
                        ot = work.tile([P, IB], f32, tag="obs", bufs=4,
                                       name=f"obs_{ib}_{co}")
                        if co == 0:
                            nc.scalar.activation(ot[:], ob[co][:], AF.Copy)
                        elif ib == NIB - 1:
                            nc.scalar.activation(ot[:], ob[co][:], AF.Copy)
                        else:
                            nc.vector.tensor_copy(ot[:], ob[co][:])
                        obs.append(ot)

                    for co in range(CC):
                        on_t = work.tile([P, IB], f32, tag="on", bufs=3, name="on_t")
                        fin = work.tile([P, IB], f32, tag="fin", bufs=3, name="fin")
                        if ib == NIB - 1:
                            # last block: co0 divides from PSUM on VectorE,
                            # co1 from SBUF on GpSimd in parallel; the two
                            # stores issue on different DMA queues
                            if co == 0:
                                nc.vector.tensor_mul(on_t[:], ob[co][:], rec[:])
                            else:
                                nc.gpsimd.tensor_mul(on_t[:], obs[co][:], rec[:])
                            nc.vector.tensor_add(fin[:], on_t[:], xt[:, co, isl])
                            if co == 0:
                                nc.sync.dma_start(out_r[:, co, isl], fin[:])
                            else:
                                nc.scalar.dma_start(out_r[:, co, isl], fin[:])
                        else:
                            nc.vector.tensor_mul(on_t[:], obs[co][:], rec[:])
                            nc.vector.tensor_add(fin[:], on_t[:], xt[:, co, isl])
                            nc.sync.dma_start(out_r[:, co, isl], fin[:])

    nc.compile()
    nc.m = get_hw_module(nc.m)
    return nc


def _get_nc():
    if "nc" not in _CACHE:
        _CACHE["nc"] = _build()
    return _CACHE["nc"]


def _prep_inputs(x, gn_scale, gn_bias, wq, bq, wk, bk, wv, bv, wo, bo):
    f = np.float32
    x = np.asarray(x, f)
    b = x.shape[0]
    scale = 1.0 / np.sqrt(np.float64(C))
    wqT = (np.asarray(wq, np.float64) * scale).T
    bq2 = (np.asarray(bq, np.float64) * scale).astype(f)
    wkT = np.asarray(wk, np.float64).T
    wovT = (np.asarray(wo, np.float64) @ np.asarray(wv, np.float64)).T
    bo2 = (np.asarray(bo, np.float64)
           + np.asarray(wo, np.float64) @ np.asarray(bv, np.float64)).astype(f)
    wstack = np.ascontiguousarray(
        np.stack([wqT, wkT, wovT]).astype(f))
    bstack = np.ascontiguousarray(np.stack(
        [bq2, np.asarray(bk, f), bo2, np.asarray(gn_scale, f),
         np.asarray(gn_bias, f)]))

    gm = np.zeros((CC, P, G), f)
    for cc in range(CC):
        for p in range(P):
            gm[cc, p, (cc * P + p) // (C // G)] = 1.0
    gmT = np.ascontiguousarray(np.transpose(gm, (0, 2, 1)))

    shared = {"wstack": wstack, "bstack": bstack, "Gm": gm, "GmT": gmT}
    in_maps = []
    for i in range(b):
        m = dict(shared)
        m["x"] = np.ascontiguousarray(x[i].reshape(C, N))
        in_maps.append(m)
    return in_maps


def _run(in_maps, trace=False, trace_cores=None):
    from concourse import bass_utils
    nc = _get_nc()
    return bass_utils.run_bass_kernel_spmd(
        nc, in_maps, core_ids=list(range(len(in_maps))),
        trace=trace, trace_cores=trace_cores)


def kernel(x, gn_scale, gn_bias, wq, bq, wk, bk, wv, bv, wo, bo):
    in_maps = _prep_inputs(x, gn_scale, gn_bias, wq, bq, wk, bk, wv, bv, wo, bo)
    res = _run(in_maps)
    b = np.asarray(x).shape[0]
    out = np.stack([res.results[i]["out"].reshape(C, H, W) for i in range(b)])
    return out.astype(np.float32)


# revision 69
# speedup vs baseline: 1.0168x; 1.0081x over previous
"""AttnBlock (GroupNorm -> QKV 1x1 -> full NxN attention -> out-proj + residual)
on 8 Trainium2 NeuronCores, data-parallel over batch (1 batch element/core).

Shapes (hardcoded): x [8, 256, 64, 64] f32, c=256, n=h*w=4096.

Per-core scheme (all on-chip, no transposes):
  - x, q, k live in [c, n] layout: SBUF [128 part, 2 c-chunks, 4096].
    Projections read an f32r rounding copy of x (the BIR verifier requires
    f32r matmul inputs from a rounding producer); GroupNorm's affine is
    folded into the weights/biases on-chip: A,B derived from bn_stats,
    folded into w and the eviction biases.
  - scores computed transposed: sT[j, i] = sum_c k[c,j] q[c,i] (lhsT=k,
    rhs=q), softmax without max-subtraction (scores ~N(0,1) after the 1/16
    scale folded into wq), exp on ScalarE PSUM->SBUF (2 j-tiles per ACT op).
  - v is produced pre-transposed with the output projection folded in:
    v'T[j, co] = sum_ci hn[ci, j] * (wo@wv).T[ci, co]  (lhsT=x, rhs=wovT),
    two j-tiles batched per PSUM bank so evictions are [128,512].
  - q is projected per i-block inside the attention loop, software-pipelined
    one block ahead so its eviction never stalls the score matmuls.
  - out[co, i] = sum_j v'T[j, co] * exp(sT)[j, i] accumulates in PSUM over j;
    softmax denominators via partial sums of exp tiles (5 groups on GpSimd,
    10 on VectorE with the GpSimd partial folded in mid-block, the last
    group fed directly), then ones[128,128] f32r matmuls that reduce over
    partitions and broadcast to 128 partitions; division by the denominator
    + residual add follow on VectorE.
  - the next block's first two score groups are pre-emitted ahead of the
    denominator matmuls so exp latency never stalls the PE at block
    boundaries.
  - biases: bq, bk applied at PSUM eviction (per-partition); bv/bo folded
    host-side into bo' = bo + wo@bv (softmax rows sum to 1), added into the
    x residual per i-block on GpSimd once q has consumed that slice.
  - PE warm-up: dummy matmuls spread across the x-DMA window (plus a batch
    riding the w_l DMA) keep the HAM clock gate at 8/8 so the projection
    matmuls run at 2.4 GHz from the start.
"""

import sys

if "/opt/trn_rl_repo" not in sys.path:
    sys.path.insert(0, "/opt/trn_rl_repo")

import numpy as np

P = 128
C = 256
CC = C // P          # 2 channel chunks
H = W = 64
N = H * W            # 4096
NJT = N // P         # 32 j-tiles of 128
IB = 512             # i-block (psum bank width)
NIB = N // IB        # 8 i-blocks
JG = 2               # j-tiles per exp group
NGRP = NJT // JG     # exp groups per i-block
G = 8                # groups
EPS = 1e-6

# x DMA chunks: uneven so the final bn_stats (on the GN critical path) is
# short
XCHUNKS = [512] * 7 + [448, 64]
XCH = len(XCHUNKS)
XOFF = [sum(XCHUNKS[:i]) for i in range(XCH + 1)]

_CACHE = {}


def _build():
    import concourse.tile as tile
    from concourse import bacc, bass_isa, mybir
    from concourse.bass_interp import get_hw_module

    f32 = mybir.dt.float32
    f32r = mybir.dt.float32r
    AF = mybir.ActivationFunctionType
    OP = mybir.AluOpType

    nc = bacc.Bacc("TRN2", target_bir_lowering=False, debug=False,
                   enable_asserts=False, num_devices=1)

    x_d = nc.dram_tensor("x", (C, N), f32, kind="ExternalInput").ap()
    ws_d = nc.dram_tensor("wstack", (3, C, C), f32, kind="ExternalInput").ap()
    bs_d = nc.dram_tensor("bstack", (5, C), f32, kind="ExternalInput").ap()
    g_d = nc.dram_tensor("Gm", (CC, P, G), f32, kind="ExternalInput").ap()
    gt_d = nc.dram_tensor("GmT", (CC, G, P), f32, kind="ExternalInput").ap()
    out_d = nc.dram_tensor("out", (C, N), f32, kind="ExternalOutput").ap()

    x_r = x_d.rearrange("(cc p) n -> p cc n", p=P)
    out_r = out_d.rearrange("(cc p) n -> p cc n", p=P)

    # index of the one ACT table set covering every func we use
    # (ln, exp, square, copy, identity)
    from concourse.hw_specs import get_activation_tables
    act_sets = list(get_activation_tables(nc.m.arch))
    LNEXP_SET = act_sets.index("natural_log_exp_and_others")

    with tile.TileContext(nc) as tc:
        with (
            tc.tile_pool(name="const", bufs=1) as const,
            tc.tile_pool(name="data", bufs=1) as data,
            tc.tile_pool(name="ps", bufs=1, space="PSUM") as ps,
        ):
            # preload it once at t=0 so no mid-kernel table switches occur
            _ld = mybir.InstLoadActFuncSet(
                name=nc.get_next_instruction_name(), ins=[], outs=[],
                act_func_set_id=LNEXP_SET)
            nc.scalar.add_instruction(_ld)

            # ---- load x in chunks first (DMA-critical), GN stats overlap ----
            xt = data.tile([P, CC, N], f32, tag="x")
            k_t = data.tile([P, CC, N], f32r, tag="k")
            vp_t = data.tile([P, NJT, C], f32r, tag="vp")
            # f32r copy of x for the projection matmuls (the BIR verifier
            # requires f32r matmul inputs to come from a rounding producer);
            # lives in the whole-kernel pool because q projects per i-block
            # inside the attention loop
            xr_t = data.tile([P, CC, N], f32r, tag="xr")

            def xr(ci, sl):
                return xr_t[:, ci, sl]

            with tc.tile_pool(name="hnp", bufs=1) as hnp:
                ones_l = hnp.tile([P, P], f32, tag="onesl")
                nc.gpsimd.memset(ones_l[:], 1.0)
                ones_t = const.tile([P, P], f32r, tag="ones")
                nc.gpsimd.tensor_copy(ones_t[:], ones_l[:])
                eps_t = const.tile([G, 1], f32, tag="eps")
                nc.gpsimd.memset(eps_t[:], EPS)

                # per-chunk bn_stats records; dummy matmuls ride the later
                # chunks to keep the PE HAM clock gate warm through the DMA
                st6 = hnp.tile([P, CC, XCH, 6], f32, tag="st6")
                warm = ps.tile([P, IB], f32, tag="proj", bufs=2, name="warm")
                for xc in range(XCH):
                    nsl = slice(XOFF[xc], XOFF[xc + 1])
                    nc.sync.dma_start(xt[:, :, nsl], x_r[:, :, nsl])
                    for cc in range(CC):
                        nc.vector.bn_stats(st6[:, cc, xc], xt[:, cc, nsl])
                        if (xc + cc) % 2 == 0:
                            nc.scalar.activation(xr_t[:, cc, nsl],
                                                 xt[:, cc, nsl], AF.Copy)
                        else:
                            nc.vector.tensor_copy(xr_t[:, cc, nsl],
                                                  xt[:, cc, nsl])
                    if xc >= 3 and XCHUNKS[xc] >= C:
                        w_n = min(IB, XCHUNKS[xc])
                        wsl = slice(XOFF[xc], XOFF[xc] + w_n)
                        for _ in range(3):
                            nc.tensor.matmul(warm[:, :w_n], ones_t[:],
                                             xr(0, wsl), start=True, stop=True)

                # ---- constants (after x on the DMA queue; small ones first
                # -- g/gt/b gate the GN chain, w_l only the later fold) ----
                b_t = const.tile([P, 5, CC], f32, tag="bt")
                nc.sync.dma_start(b_t[:], bs_d.rearrange("v (cc p) -> p v cc", p=P))
                g_t = const.tile([P, CC, G], f32, tag="G")
                nc.sync.dma_start(g_t[:], g_d.rearrange("cc p g -> p cc g"))
                gt_t = const.tile([G, CC, P], f32, tag="GT")
                nc.sync.dma_start(gt_t[:], gt_d.rearrange("cc g p -> g cc p"))
                w_l = hnp.tile([P, 3, CC, C], f32, tag="wl")
                nc.sync.dma_start(w_l[:], ws_d.rearrange("w (cc p) o -> p w cc o", p=P))

                w_r = const.tile([P, 3, CC, C], f32r, tag="wr")
                wq_t, wk_t, wov_t = w_r[:, 0], w_r[:, 1], w_r[:, 2]
                bq_t, bk_t, bo_t, gns_t, gnb_t = (b_t[:, v] for v in range(5))

                # bridge the PE warm-up across the stats->fold serial chain:
                # these ride the w_l DMA (~16.6us) so the HAM window never
                # sees >3.4us of PE idle before the k projection starts
                wlr = hnp.tile([P, CC, C], f32r, tag="wlr")
                nc.gpsimd.tensor_copy(wlr[:], w_l[:, 0])
                for _ in range(3):
                    nc.tensor.matmul(warm[:], ones_t[:], wlr[:],
                                     start=True, stop=True)

                # per-channel (mean, E[x^2]) from aggregated bn records;
                # cc0 on VectorE, cc1 on GpSimd to shorten the serial chain
                mv = const.tile([P, CC, 2], f32, tag="mv")
                stc = const.tile([P, CC, 2], f32, tag="stc")
                for cc in range(CC):
                    nc.vector.bn_aggr(mv[:, cc], st6[:, cc])
                    eng = nc.vector if cc == 0 else nc.gpsimd
                    eng.tensor_copy(stc[:, cc, 0:1], mv[:, cc, 0:1])
                    eng.tensor_mul(stc[:, cc, 1:2], mv[:, cc, 0:1],
                                   mv[:, cc, 0:1])
                    eng.tensor_add(stc[:, cc, 1:2], stc[:, cc, 1:2],
                                   mv[:, cc, 1:2])

                # group-reduce per-channel (mean, E[x^2]) straight in PSUM
                # (score-tag slot: keeps both proj slots free for chps)
                gps = ps.tile([G, 2], f32, tag="score", bufs=2, name="gps")
                for cc in range(CC):
                    nc.tensor.matmul(gps[:], g_t[:, cc], stc[:, cc],
                                     start=(cc == 0), stop=(cc == CC - 1))

                # grp cols: 0=mean 1=rstd 2=ex2 3=mean^2 4=var 5=sqrt(var+eps)
                CPG = C // G
                grp = const.tile([G, 6], f32, tag="grp")
                nc.vector.tensor_scalar_mul(grp[:, 0:1], gps[:, 0:1], 1.0 / CPG)
                nc.vector.tensor_scalar_mul(grp[:, 2:3], gps[:, 1:2], 1.0 / CPG)
                nc.vector.tensor_mul(grp[:, 3:4], grp[:, 0:1], grp[:, 0:1])
                nc.vector.tensor_sub(grp[:, 4:5], grp[:, 2:3], grp[:, 3:4])
                # rstd = exp(-0.5*ln(var+eps)): keeps every ACT func in the
                # natural_log_exp set -> a single table load for the kernel
                nc.scalar.activation(grp[:, 5:6], grp[:, 4:5], AF.Ln, bias=eps_t[:])
                nc.scalar.activation(grp[:, 1:2], grp[:, 5:6], AF.Exp, scale=-0.5)

                # broadcast group (mean, rstd) to channels; A/B affine coeffs
                ab = const.tile([P, CC, 2], f32, tag="ab")  # 0=A 1=B
                for cc in range(CC):
                    chps = ps.tile([P, 2], f32, tag="proj", bufs=2, name="chps")
                    nc.tensor.matmul(chps[:], gt_t[:, cc], grp[:, 0:2],
                                     start=True, stop=True)
                    # A = rstd * gn_scale
                    nc.vector.tensor_mul(ab[:, cc, 0:1], chps[:, 1:2],
                                         gns_t[:, cc:cc + 1])
                    # B = gn_bias - mean * A
                    nc.vector.tensor_mul(ab[:, cc, 1:2], chps[:, 0:1], ab[:, cc, 0:1])
                    nc.vector.tensor_sub(ab[:, cc, 1:2], gnb_t[:, cc:cc + 1],
                                         ab[:, cc, 1:2])

                # fold GN affine into the weights: w'[ci,:] = w[ci,:]*A[ci]
                # (k on VectorE -- it gates the first projection; wov/q on
                # GpSimd so VectorE is free for the first k evictions)
                for w in (1, 2, 0):
                    for cc in range(CC):
                        eng = nc.vector if w == 1 else nc.gpsimd
                        eng.tensor_scalar_mul(w_r[:, w, cc],
                                              w_l[:, w, cc],
                                              ab[:, cc, 0:1])

                # fold the w^T @ B terms into the eviction biases (fp32,
                # N=1 matmuls); for wov the term rides the residual bias
                # because softmax rows sum to 1
                bq3 = const.tile([P, CC], f32, tag="bq3")
                bk3 = const.tile([P, CC], f32, tag="bk3")
                bo3 = const.tile([P, CC], f32, tag="bo3")
                for w, (b_in, b_out) in ((1, (bk_t, bk3)), (0, (bq_t, bq3)),
                                         (2, (bo_t, bo3))):
                    for oc in range(CC):
                        bp = ps.tile([P, 1], f32, tag="proj", bufs=2, name="bp")
                        for ci in range(CC):
                            nc.tensor.matmul(bp[:],
                                             w_l[:, w, ci, oc * P:(oc + 1) * P],
                                             ab[:, ci, 1:2],
                                             start=(ci == 0), stop=(ci == CC - 1))
                        nc.vector.tensor_add(b_out[:, oc:oc + 1], bp[:],
                                             b_in[:, oc:oc + 1])

                def emit_q(ib):
                    isl = slice(ib * IB, (ib + 1) * IB)
                    qt = data.tile([P, CC, IB], f32r, tag="qt", bufs=2,
                                   name=f"qt{ib}")
                    for oc in range(CC):
                        pq = ps.tile([P, IB], f32, tag="proj", bufs=2, name="pq")
                        for ci in range(CC):
                            nc.tensor.matmul(
                                pq[:],
                                wq_t[:, ci, oc * P:(oc + 1) * P],
                                xr(ci, isl),
                                start=(ci == 0), stop=(ci == CC - 1))
                        if oc == 0:
                            nc.vector.tensor_scalar_add(qt[:, oc], pq[:],
                                                        bq3[:, oc:oc + 1])
                        else:
                            nc.scalar.activation(qt[:, oc], pq[:], AF.Identity,
                                                 bias=bq3[:, oc:oc + 1])
                    return qt

                # ---- k projection (evictions alternate ScalarE/VectorE) ----
                for oc in range(CC):
                    for ib in range(NIB):
                        isl = slice(ib * IB, (ib + 1) * IB)
                        it = oc * NIB + ib
                        # rotate across both PSUM tags for a 4-slot pipeline
                        if it % 2 == 0:
                            pp = ps.tile([P, IB], f32, tag="proj", bufs=2,
                                         name="pp")
                        else:
                            pp = ps.tile([P, JG, IB], f32, tag="score", bufs=2,
                                         name="pps")[:, 0]
                        for ci in range(CC):
                            nc.tensor.matmul(
                                pp,
                                wk_t[:, ci, oc * P:(oc + 1) * P],
                                xr(ci, isl),
                                start=(ci == 0), stop=(ci == CC - 1))
                        if it % 2 == 0:
                            nc.scalar.activation(k_t[:, oc, isl], pp,
                                                 AF.Identity,
                                                 bias=bk3[:, oc:oc + 1])
                        else:
                            nc.vector.tensor_scalar_add(k_t[:, oc, isl], pp,
                                                        bk3[:, oc:oc + 1])

                # ---- v'T[j, co] = sum_ci hn[ci, j] wovT[ci, co] (no bias),
                # two j-tiles per PSUM bank, evictions alternate engines;
                # q for the first i-block projects mid-phase so its eviction
                # clears the queues before the first score matmuls ----
                qts = {}
                for jp in range(NJT // 2):
                    if jp == NJT // 2 - 4:
                        qts[0] = emit_q(0)
                    if jp % 2 == 0:
                        pv = ps.tile([P, 2, C], f32, tag="proj", bufs=2,
                                     name="pv")[:]
                    else:
                        pv = ps.tile([P, JG, IB], f32, tag="score", bufs=2,
                                     name="pvs")[:, :, 0:C]
                    for t in range(2):
                        jt = 2 * jp + t
                        for ci in range(CC):
                            nc.tensor.matmul(
                                pv[:, t],
                                xr(ci, slice(jt * P, (jt + 1) * P)),
                                wov_t[:, ci, :],
                                start=(ci == 0), stop=(ci == CC - 1))
                    if jp % 2 == 0:
                        nc.vector.tensor_copy(vp_t[:, 2 * jp:2 * jp + 2], pv)
                    else:
                        nc.scalar.activation(vp_t[:, 2 * jp:2 * jp + 2], pv,
                                             AF.Copy)

            with tc.tile_pool(name="work", bufs=1) as work:
                def emit_scores(qt, g):
                    ssg = ps.tile([P, JG, IB], f32, tag="score", bufs=2,
                                  name="ssg")
                    for t in range(JG):
                        jt = g * JG + t
                        for ci in range(CC):
                            nc.tensor.matmul(
                                ssg[:, t],
                                k_t[:, ci, jt * P:(jt + 1) * P],
                                qt[:, ci, :],
                                start=(ci == 0), stop=(ci == CC - 1))
                    return ssg

                pre = None
                for ib in range(NIB):
                    isl = slice(ib * IB, (ib + 1) * IB)
                    qt = qts.pop(ib)
                    # residual base: xt += bo3 for this i-block (safe: q for
                    # this block was projected in the previous iteration)
                    for co in range(CC):
                        nc.gpsimd.tensor_scalar_add(xt[:, co, isl],
                                                    xt[:, co, isl],
                                                    bo3[:, co:co + 1])
                    ob = []
                    for co in range(CC):
                        obt = ps.tile([P, IB], f32, tag="ob", bufs=2,
                                      name=f"ob_{ib}_{co}")
                        ob.append(obt)
                    esa = work.tile([P, JG, IB], f32r, tag="esum", bufs=3,
                                    name="esa")
                    esb = work.tile([P, JG, IB], f32r, tag="esum", bufs=3,
                                    name="esb")

                    # two score groups run ahead so the PE stream never has
                    # an ob matmul queued head-of-line behind an unfinished
                    # exp; for ib>0 they were pre-emitted before the previous
                    # block's denominator matmuls
                    if pre is None:
                        ssgs = {0: emit_scores(qt, 0), 1: emit_scores(qt, 1)}
                    else:
                        ssgs = dict(enumerate(pre))
                    for g in range(NGRP):
                        ssg = ssgs.pop(g)
                        et = work.tile([P, JG, IB], f32r, tag="exp", bufs=4,
                                       name="et")
                        if isinstance(ssg, list):
                            for t in range(JG):
                                nc.scalar.activation(et[:, t], ssg[t][:],
                                                     AF.Exp)
                        else:
                            nc.scalar.activation(et[:], ssg[:], AF.Exp)
                        if g == 0 and ib + 1 < NIB:
                            # project q for the next i-block now; its eviction
                            # completes long before that block's scores start
                            qts[ib + 1] = emit_q(ib + 1)
                        if g + 2 < NGRP and g + 2 not in ssgs:
                            ssgs[g + 2] = emit_scores(qt, g + 2)
                        if g == NGRP - 1:
                            # fold the esa column-halves on VectorE (off the
                            # tail: its input chain ended at g-1) so the
                            # partition-reduce needs fewer matmuls
                            esaT = work.tile([P, IB], f32r, tag="esat",
                                             bufs=2, name="esaT")
                            nc.vector.tensor_add(esaT[:],
                                                 esa[:, 0].bitcast(f32),
                                                 esa[:, 1].bitcast(f32))
                            # denominators FIRST: they and the ob group
                            # below both wait on this exp, so ordering the
                            # denominator ahead lets the reciprocal run
                            # during the final ob matmuls -- off the tail
                            if ib < NIB - 1:
                                smr = work.tile([P, IB], f32, tag="smr",
                                                bufs=2, name="smr")
                                # mid-kernel blocks have ~28us of tail slack:
                                # fold this exp group's halves on VectorE too
                                # and reduce with a single ones-matmul
                                nc.vector.tensor_add(esaT[:],
                                                     esaT[:].bitcast(f32),
                                                     et[:, 0].bitcast(f32))
                                nc.vector.tensor_add(esaT[:],
                                                     esaT[:].bitcast(f32),
                                                     et[:, 1].bitcast(f32))
                                # GpSimd all-reduce does reduce+broadcast off
                                # the PE entirely; its latency rides the
                                # ~28us mid-block tail slack (bank release
                                # comes from the obs copies, not this path)
                                nc.gpsimd.partition_all_reduce(
                                    smr[:], esaT[:].bitcast(f32), P,
                                    bass_isa.ReduceOp.add)
                            else:
                                # final block: exp latency must not extend the
                                # exposed tail -- feed its halves directly
                                # to PE ones-matmuls (PSUM)
                                smr = ps.tile([P, IB], f32, tag="proj",
                                              bufs=2, name="smt")
                                nc.tensor.matmul(smr[:], ones_t[:], esaT[:],
                                                 start=True, stop=False)
                                nc.tensor.matmul(smr[:], ones_t[:], et[:, 0],
                                                 start=False, stop=False)
                                nc.tensor.matmul(smr[:], ones_t[:], et[:, 1],
                                                 start=False, stop=True)
                            rec = work.tile([P, IB], f32, tag="rec", bufs=2,
                                            name="rec")
                            nc.vector.reciprocal(rec[:], smr[:])
                        for t in range(JG):
                            jt = g * JG + t
                            for co in range(CC):
                                nc.tensor.matmul(
                                    ob[co][:],
                                    vp_t[:, jt, co * P:(co + 1) * P],
                                    et[:, t],
                                    start=(jt == 0), stop=(jt == NJT - 1))
                        # partial-sum split: GpSimd is ~2x slower per add than
                        # VectorE, so it gets 5 groups and VectorE 10; the
                        # last group feeds the denominator matmuls directly
                        if g == NGRP - 1:
                            pass
                        elif g == 0:
                            nc.gpsimd.tensor_copy(esb[:], et[:].bitcast(f32))
                        elif g < 5:
                            nc.gpsimd.tensor_add(esb[:], esb[:].bitcast(f32),
                                                 et[:].bitcast(f32))
                        elif g == 5:
                            nc.vector.tensor_copy(esa[:], et[:].bitcast(f32))
                        else:
                            nc.vector.tensor_add(esa[:], esa[:].bitcast(f32),
                                                 et[:].bitcast(f32))
                            if g == 10:
                                # fold the (complete) GpSimd partial into the
                                # VectorE chain here, well off the tail path
                                nc.vector.tensor_add(esa[:],
                                                     esa[:].bitcast(f32),
                                                     esb[:].bitcast(f32))

                    # pre-emit the next block's first two score groups:
                    # exp(0') latency then hides behind this block's tail
                    # instead of stalling the PE.  The final block gets a
                    # third group (split over the two proj-tag banks) because
                    # it has no next-q matmuls to pad the runway with.
                    if ib + 1 < NIB:
                        nqt = qts[ib + 1]
                        pre = [emit_scores(nqt, 0), emit_scores(nqt, 1)]
                        if ib + 1 == NIB - 1:
                            g2 = []
                            for t in range(JG):
                                jt = 2 * JG + t
                                s1 = ps.tile([P, IB], f32, tag="proj", bufs=2,
                                             name="pre2")
                                for ci in range(CC):
                                    nc.tensor.matmul(
                                        s1[:],
                                        k_t[:, ci, jt * P:(jt + 1) * P],
                                        nqt[:, ci, :],
                                        start=(ci == 0), stop=(ci == CC - 1))
                                g2.append(s1)
                            pre.append(g2)
                    else:
                        pre = None

                    # free the ob psum slots right away (one copy per engine);
                    # the last block divides co0 straight from PSUM instead
                    # (no successor needs the bank), keeping its tail short
                    obs = []
                    for co in range(CC):
                        if ib == NIB - 1 and co == 0:
                            obs.append(None)
                            continue
                        ot = work.tile([P, IB], f32, tag="obs", bufs=4,
                                       name=f"obs_{ib}_{co}")
                        if co == 0:
                            nc.scalar.activation(ot[:], ob[co][:], AF.Copy)
                        elif ib == NIB - 1:
                            nc.scalar.activation(ot[:], ob[co][:], AF.Copy)
                        else:
                            nc.vector.tensor_copy(ot[:], ob[co][:])
                        obs.append(ot)

                    for co in range(CC):
                        on_t = work.tile([P, IB], f32, tag="on", bufs=3, name="on_t")
                        fin = work.tile([P, IB], f32, tag="fin", bufs=3, name="fin")
                        if ib == NIB - 1:
                            # last block: co0 divides from PSUM on VectorE,
                            # co1 from SBUF on GpSimd in parallel; the two
                            # stores issue on different DMA queues
                            if co == 0:
                                nc.vector.tensor_mul(on_t[:], ob[co][:], rec[:])
                            else:
                                nc.gpsimd.tensor_mul(on_t[:], obs[co][:], rec[:])
                            nc.vector.tensor_add(fin[:], on_t[:], xt[:, co, isl])
                            if co == 0:
                                nc.sync.dma_start(out_r[:, co, isl], fin[:])
                            else:
                                nc.scalar.dma_start(out_r[:, co, isl], fin[:])
                        else:
                            nc.vector.tensor_mul(on_t[:], obs[co][:], rec[:])
                            nc.vector.tensor_add(fin[:], on_t[:], xt[:, co, isl])
                            nc.sync.dma_start(out_r[:, co, isl], fin[:])

    nc.compile()
    nc.m = get_hw_module(nc.m)
    return nc


def _get_nc():
    if "nc" not in _CACHE:
        _CACHE["nc"] = _build()
    return _CACHE["nc"]


def _prep_inputs(x, gn_scale, gn_bias, wq, bq, wk, bk, wv, bv, wo, bo):
    f = np.float32
    x = np.asarray(x, f)
    b = x.shape[0]
    scale = 1.0 / np.sqrt(np.float64(C))
    wqT = (np.asarray(wq, np.float64) * scale).T
    bq2 = (np.asarray(bq, np.float64) * scale).astype(f)
    wkT = np.asarray(wk, np.float64).T
    wovT = (np.asarray(wo, np.float64) @ np.asarray(wv, np.float64)).T
    bo2 = (np.asarray(bo, np.float64)
           + np.asarray(wo, np.float64) @ np.asarray(bv, np.float64)).astype(f)
    wstack = np.ascontiguousarray(
        np.stack([wqT, wkT, wovT]).astype(f))
    bstack = np.ascontiguousarray(np.stack(
        [bq2, np.asarray(bk, f), bo2, np.asarray(gn_scale, f),
         np.asarray(gn_bias, f)]))

    gm = np.zeros((CC, P, G), f)
    for cc in range(CC):
        for p in range(P):
            gm[cc, p, (cc * P + p) // (C // G)] = 1.0
    gmT = np.ascontiguousarray(np.transpose(gm, (0, 2, 1)))

    shared = {"wstack": wstack, "bstack": bstack, "Gm": gm, "GmT": gmT}
    in_maps = []
    for i in range(b):
        m = dict(shared)
        m["x"] = np.ascontiguousarray(x[i].reshape(C, N))
        in_maps.append(m)
    return in_maps


def _run(in_maps, trace=False, trace_cores=None):
    from concourse import bass_utils
    nc = _get_nc()
    return bass_utils.run_bass_kernel_spmd(
        nc, in_maps, core_ids=list(range(len(in_maps))),
        trace=trace, trace_cores=trace_cores)


def kernel(x, gn_scale, gn_bias, wq, bq, wk, bk, wv, bv, wo, bo):
    in_maps = _prep_inputs(x, gn_scale, gn_bias, wq, bq, wk, bk, wv, bv, wo, bo)
    res = _run(in_maps)
    b = np.asarray(x).shape[0]
    out = np.stack([res.results[i]["out"].reshape(C, H, W) for i in range(b)])
    return out.astype(np.float32)
